# revision 4
# baseline (speedup 1.0000x reference)
"""Trainium2 Bass kernel for NetTGCN (gnn_message_passing).

Strategy
--------
Dense Chebyshev SpMM: the normalized adjacency LhatT is column-sharded
across the 8 cores; each core holds lhsT [8192, 1024] fp16 SBUF-resident
and computes its 1024 rows of every Lhat application.  To halve the
host->device upload, the adjacency travels as the int8 edge-count matrix
A plus dinv vectors; the device rebuilds LhatT = -dinv_i A dinv_j by
scaling rows during the int8->fp16 conversion and folding the column
scale (and minus sign) into the matmul PSUM output.  The full x state is
likewise assembled on-device by AllGathering the per-core shards.

The z state (batches folded into the matmul free dim) is re-assembled
after every Lhat application with an AllGather whose staging buffers are
partition-major (multi-KB contiguous DMA runs).  conv1 runs one 240-wide
recursion (B*T); conv2 two sequential 256-wide chunk recursions.  Per
k-step the AllGather is split into two mt-halves: the first flies under
the back half of the matmul phase and its readback lands in a ping-pong
buffer (Za) with no WAR hazard, so the next k-step's half-0 matmuls plus
the per-tap GEMMs cover the second collective's latency.  The DFT (real
part of a length-15 DFT) is folded into W1 on the host.

fc1 is contraction-sharded: each core streams 1/8 of the weight from HBM
as int8 (per-output-channel scales folded into fc2's weights host-side),
converts to fp16 on the vector engine, accumulates [B, C] partials in
PSUM, then AllReduces; fc2 + log_softmax run replicated on every core.

fp16 matmul operands, fp32 PSUM accumulation.
"""

import numpy as np

# ---------------------------------------------------------------- config


class CFG:
    N = 8192
    B = 16
    T = 15
    KCH = 25
    G1 = 32
    G2 = 64
    C = 512
    D = 6
    NCORES = 8
    NLOC = 1024
    MT = 8           # local 128-row m-tiles
    NBLK = 64        # global 128-row blocks
    F1 = 240         # B*T
    F2 = 256         # conv2 chunk width (8 batches * G1)
    PHASES = 3       # 1=conv1, 2=+conv2, 3=+fc
    DEBUG = False


def _host_prep(cfg, x, edge_index, W1, b1, W2, b2, fc1_w, fc1_b, fc2_w, fc2_b):
    """Pure layout / format preprocessing -> per-core input maps."""
    f16 = np.float16
    N, B, T, K = cfg.N, cfg.B, cfg.T, cfg.KCH
    NC, NLOC, MT = cfg.NCORES, cfg.NLOC, cfg.MT
    G1, G2, C = cfg.G1, cfg.G2, cfg.C

    row = np.asarray(edge_index[0], dtype=np.int64)
    col = np.asarray(edge_index[1], dtype=np.int64)
    deg = np.bincount(row, minlength=N).astype(np.float32)
    dinv = np.where(deg > 0, 1.0 / np.sqrt(np.maximum(deg, 1.0)), 0.0).astype(np.float32)
    # Lhat is uploaded as the int8 edge-count matrix A (half the bytes of
    # fp16); the device rebuilds LhatT[i,j] = -dinv_i * dinv_j * A[i,j] by
    # scaling rows during int8->fp16 conversion (dinvcol) and columns on
    # the matmul output (dscale, applied to PSUM before the recombine).
    A = np.zeros((N, N), np.int8)
    np.add.at(A, (row, col), 1)
    dinvcol = np.ascontiguousarray(dinv.reshape(cfg.NBLK, 128).T)  # [128, 64]

    # x node-major: node n = b*128 + p ; col = batch*T + t
    x_n = np.ascontiguousarray(
        np.asarray(x, np.float32).transpose(1, 0, 2).reshape(cfg.NBLK, 128, B * T)
        .transpose(1, 0, 2)).astype(f16)                      # [128, 64, 240]

    # fold DFT-real (cosine) matrix into W1:  xf = x @ Cf ; W1f[k] = Cf @ W1[k]
    tt = np.arange(T)
    Cf = np.cos(2 * np.pi * np.outer(tt, tt) / T).astype(np.float32)
    W1f = np.einsum('ts,ksg->ktg', Cf, np.asarray(W1, np.float32))  # [K, T, G1]

    # conv1 tap weights: 8-batch block-diag [K, 120, 256] -> sbuf [120, K, 256]
    W1blk = np.zeros((K, 8 * T, 8 * G1), np.float32)
    for b8 in range(8):
        W1blk[:, b8 * T:(b8 + 1) * T, b8 * G1:(b8 + 1) * G1] = W1f
    W1sb = np.ascontiguousarray(W1blk.transpose(1, 0, 2)).astype(f16)

    # conv2 tap weights: 4-batch block-diag [K, 128, 256] -> sbuf kept in DRAM,
    # streamed per k as [128, 256]
    W2blk = np.zeros((K, 4 * G1, 4 * G2), np.float32)
    for b4 in range(4):
        W2blk[:, b4 * G1:(b4 + 1) * G1, b4 * G2:(b4 + 1) * G2] = \
            np.asarray(W2, np.float32)
    W2blk_d = np.ascontiguousarray(W2blk).astype(f16)         # [K, 128, 256]

    b1row = np.tile(np.asarray(b1, np.float32), B)[None, :].astype(f16)   # [1, 512]
    b2row = np.tile(np.asarray(b2, np.float32), B)[None, :].astype(f16)   # [1, 1024]
    ones_col = np.ones((1, 128), f16)

    # fc1 weights int8 with per-output-channel scale; the scale (and the
    # scaled-down bias) are folded into fc2's weight / fc1's bias on the host
    w = np.asarray(fc1_w, np.float32)                          # [C, N*G2]
    wscale = np.maximum(np.abs(w).max(axis=1), 1e-30) / 127.0  # [C]
    w8 = np.clip(np.round(w / wscale[:, None]), -127, 127).astype(np.int8)
    fc1b_row = (np.asarray(fc1_b, np.float32) / wscale)[None, :].astype(f16)
    fc2_wT = np.ascontiguousarray(
        (np.asarray(fc2_w, np.float32) * wscale[None, :])
        .T.reshape(cfg.C // 128, 128, cfg.D)
        .transpose(1, 0, 2))                                   # [128, C/128, D] f32
    fc2b_col = np.asarray(fc2_b, np.float32)[None, :]          # [1, D]
    ones_f32 = np.ones((1, cfg.B), np.float32)

    wv = w8.reshape(cfg.C, N, cfg.G2)
    A_b = A.reshape(cfg.NBLK, 128, cfg.NBLK, 128)              # [bi, p, bj, m]

    in_maps = []
    for c in range(NC):
        # A column slice, global block order -> [p, u, ko, mt, m] int8
        lt = A_b[:, :, c * MT:(c + 1) * MT, :]                 # [64, 128, 8, 128]
        lt = np.ascontiguousarray(
            lt.transpose(1, 0, 2, 3).reshape(128, cfg.NBLK // 2, 2, MT, 128))
        dscale = np.ascontiguousarray(
            -dinv[c * NLOC:(c + 1) * NLOC].reshape(MT, 128).T)  # [128, 8]
        # x local rows (this core's 8 blocks)
        xloc = np.ascontiguousarray(x_n[:, c * MT:(c + 1) * MT, :])  # [128, 8, 240]
        # fc1 weight slice -> [p, jt, cc] with jt = g*MT + mt, j = jt*128 + p
        ws = wv[:, c * NLOC:(c + 1) * NLOC, :]                 # [C, NLOC, G2] int8
        ws = ws.reshape(cfg.C, MT, 128, cfg.G2).transpose(2, 3, 1, 0)
        ws = np.ascontiguousarray(ws.reshape(128, cfg.G2 * MT, cfg.C))
        in_maps.append(dict(
            lt=lt, dinvcol=dinvcol, dscale=dscale, x_loc=xloc,
            w1sb=W1sb, w2blk=W2blk_d, b1row=b1row, b2row=b2row,
            ones16=ones_col, fc1b=fc1b_row, fc2wt=fc2_wT, fc2b=fc2b_col,
            onesf32=ones_f32, wfc=ws,
        ))
    return in_maps


def _build(cfg):
    import concourse.bass as bass
    import concourse.mybir as mybir
    import concourse.tile as tile
    from concourse import bacc
    from concourse.masks import make_identity

    f16 = mybir.dt.float16
    f32 = mybir.dt.float32
    AT = mybir.ActivationFunctionType
    OP = mybir.AluOpType
    AX = mybir.AxisListType

    N, B, T, K = cfg.N, cfg.B, cfg.T, cfg.KCH
    NC, MT, NBLK = cfg.NCORES, cfg.MT, cfg.NBLK
    F1, F2 = cfg.F1, cfg.F2
    G1, G2, C, D = cfg.G1, cfg.G2, cfg.C, cfg.D
    RG = [list(range(NC))]

    nc = bacc.Bacc("TRN2", target_bir_lowering=False, debug=False,
                   num_devices=NC)

    i8 = mybir.dt.int8
    dt_in = {
        'lt': ([128, NBLK // 2, 2, MT, 128], i8),
        'dinvcol': ([128, NBLK], f32),
        'dscale': ([128, MT], f32),
        'x_loc': ([128, MT, F1], f16),
        'w1sb': ([8 * T, K, 8 * G1], f16),
        'w2blk': ([K, 4 * G1, 4 * G2], f16),
        'b1row': ([1, B * G1], f16),
        'b2row': ([1, B * G2], f16),
        'ones16': ([1, 128], f16),
        'fc1b': ([1, C], f16),
        'fc2wt': ([128, C // 128, D], f32),
        'fc2b': ([1, D], f32),
        'onesf32': ([1, B], f32),
        'wfc': ([128, G2 * MT, C], i8),
    }
    din = {k: nc.dram_tensor(k, shp, dt, kind="ExternalInput").ap()
           for k, (shp, dt) in dt_in.items()}
    dout = nc.dram_tensor("out", [B, D], f32, kind="ExternalOutput").ap()
    if cfg.DEBUG:
        dbg_t1 = nc.dram_tensor("dbg_t1", [128, MT, F1], f16,
                                kind="ExternalOutput").ap()
        dbg_h1 = nc.dram_tensor("dbg_h1", [128, MT, B * G1], f16,
                                kind="ExternalOutput").ap()
        dbg_h2 = nc.dram_tensor("dbg_h2", [128, MT, B * G2], f16,
                                kind="ExternalOutput").ap()

    with tile.TileContext(nc) as tc:
        with (
            tc.tile_pool(name="const", bufs=1) as constp,
            tc.tile_pool(name="dram", bufs=1, space="DRAM") as dramp,
        ):
            # ---------------- constants / persistent state
            LT = constp.tile([128, NBLK // 2, 2, MT, 128], f16)
            dscale = constp.tile([128, MT], f32)
            nc.sync.dma_start(dscale[:], din['dscale'])
            ident16 = constp.tile([128, 128], f16)
            make_identity(nc, ident16[:])
            identf32 = constp.tile([32, 32], f32)
            make_identity(nc, identf32[:])
            ones16 = constp.tile([1, 128], f16)
            nc.sync.dma_start(ones16[:], din['ones16'])

            # rebuild LhatT fp16 from the int8 edge-count upload:
            # LT[:, u, ko, :, :] = A8 * dinv(source row)   (column scale and
            # the minus sign are applied later on the matmul PSUM output)
            with (
                tc.tile_pool(name="a8p", bufs=2) as a8p,
                tc.tile_pool(name="dcp", bufs=1) as dcp,
            ):
                dinvcol = dcp.tile([128, NBLK], f32)
                nc.sync.dma_start(dinvcol[:], din['dinvcol'])
                for uc in range(8):
                    a8b = a8p.tile([128, 4, 2, MT, 128], i8, tag="a8b")
                    nc.sync.dma_start(a8b[:],
                                      din['lt'][:, uc * 4:(uc + 1) * 4])
                    for j in range(4):
                        for ko in range(2):
                            b = (uc * 4 + j) * 2 + ko
                            nc.vector.tensor_single_scalar(
                                LT[:, uc * 4 + j, ko, :, :],
                                a8b[:, j, ko, :, :],
                                dinvcol[:, b:b + 1], OP.mult)

            h1_dram = dramp.tile([NC * 128, MT, B * G1], f16,
                                 addr_space="Shared")        # gathered h1
            gh1 = dramp.tile([128, MT, B * G1], f16)         # own h1 staging

            # ---- shared per-layer chebyshev machinery ------------------
            def cheb_chunk(F, Za, Zb, znl, wtap_ap_fn, twidth, brow,
                           bofs, acc, accw, psz, pst, psg, trsb, gin,
                           gout_fn, ghalf):
                """One chebyshev recursion of width F.
                Za: 2x [128, NBLK/2, F] ping-pong for gather-half-0 blocks
                    (block (r, m<4) at index r*4+m); readback#0 lands in the
                    *other* Za buffer mid-MM-phase (no WAR), so the next
                    k-step's half-0 matmuls can cover the AllGather#2 tail.
                Zb: [128, NBLK/2, F] half-1 blocks (r, m>=4), updated in
                    place after the MM phase.
                znl:  2x [128, MT, F] local T ping-pong; znl[0] holds T_0
                wtap_ap_fn(k): moving tap weight AP [twidth, accw//nq] per tap
                twidth: tap transpose width (<=120)
                brow/bofs: bias row AP + column offset
                acc: [128, MT, accw] accumulator; nq = F // twidth quads
                gin/gout: staging DRAM tiles, ghalf = MT // 2
                """
                nq = F // twidth
                ocols = accw // nq

                def tap(kk, src, add_bias):
                    # src: [128, MT, F] local T_kk ; out += T W
                    for mt in range(MT):
                        pg = psg.tile([128, accw], f32, tag="pg")
                        for q in range(nq):
                            tp = pst.tile([128, 128], f16, tag="tp")
                            nc.tensor.transpose(
                                tp[:twidth, :],
                                src[:, mt, q * twidth:(q + 1) * twidth],
                                ident16[:])
                            tsb = trsb.tile([128, 128], f16, tag="tsb")
                            nc.any.tensor_copy(tsb[:twidth, :], tp[:twidth, :])
                            nc.tensor.matmul(
                                pg[:, q * ocols:(q + 1) * ocols],
                                tsb[:twidth, :], wtap_ap_fn(kk),
                                start=True, stop=not add_bias)
                            if add_bias:
                                nc.tensor.matmul(
                                    pg[:, q * ocols:(q + 1) * ocols],
                                    ones16[:1, :128],
                                    brow[:1, bofs + q * ocols:
                                         bofs + (q + 1) * ocols],
                                    start=False, stop=True)
                        nc.vector.tensor_tensor(
                            acc[:, mt, bofs:bofs + accw],
                            acc[:, mt, bofs:bofs + accw], pg[:], OP.add)

                # half-0 block-pairs first: their data (Za ping-pong) is
                # ready at phase start, covering AllGather#2 + readback#1
                # which are still landing in Zb
                u_order = [u for u in range(NBLK // 2) if u % 4 < 2] + \
                          [u for u in range(NBLK // 2) if u % 4 >= 2]

                def rhs_pair(u, kk):
                    r, j = u // 4, u % 4
                    if j < 2:
                        return Za[kk % 2][:, r * 4 + 2 * j:
                                          r * 4 + 2 * j + 2, :]
                    return Zb[:, r * 4 + 2 * (j - 2):
                              r * 4 + 2 * (j - 2) + 2, :]

                tap(0, znl[0][:], False)
                for kk in range(K - 1):              # produces T_{kk+1}
                    # znl[(kk+1) % 2] holds T_{kk-1}; overwritten in place
                    cur = znl[(kk + 1) % 2]
                    for mt in range(MT):
                        ps = psz.tile([128, F], f32, tag="psz")
                        for i, u in enumerate(u_order):
                            pair = rhs_pair(u, kk)
                            nc.tensor.matmul(
                                ps[:], LT[:, u, 0, mt, :], pair[:, 0, :],
                                start=(i == 0), stop=False)
                            nc.tensor.matmul(
                                ps[:], LT[:, u, 1, mt, :], pair[:, 1, :],
                                start=False, stop=(i == NBLK // 2 - 1))
                        # fold the output-side dinv column scale (and the
                        # Lhat minus sign) into the PSUM before recombining
                        nc.vector.tensor_single_scalar(
                            ps[:], ps[:], dscale[:, mt:mt + 1], OP.mult)
                        if kk == 0:
                            nc.vector.tensor_copy(cur[:, mt, :], ps[:])
                        else:
                            nc.vector.scalar_tensor_tensor(
                                cur[:, mt, :], ps[:], 2.0, cur[:, mt, :],
                                OP.mult, OP.subtract)
                        # stage+gather each half as soon as its mts are done
                        # (the AllGather flies under the remaining matmuls)
                        if (mt == ghalf - 1 or mt == MT - 1) and kk < K - 2:
                            h0 = 0 if mt == ghalf - 1 else ghalf
                            gi = gin[h0 // ghalf]
                            nc.sync.dma_start(gi[:],
                                              cur[:, h0:h0 + ghalf, :])
                            nc.gpsimd.collective_compute(
                                "AllGather", OP.bypass, replica_groups=RG,
                                ins=[gi[:]],
                                outs=[gout_fn(kk, h0 // ghalf)[:]])
                    # taps are issued BEFORE the readbacks: the tap copies
                    # share the ACT sequencer with readback DMAs, and a
                    # readback's wait on AllGather#2 must not block them
                    tap(kk + 1, cur, add_bias=(kk == K - 2))
                    if kk < K - 2:
                        # readback#0 on the ACT HWDGE ring (not FIFO-queued
                        # behind the stage DMAs on the sync ring) -> the
                        # other Za buffer; no WAR, lands mid-MM-phase
                        nc.scalar.dma_start(
                            Za[(kk + 1) % 2][:]
                            .rearrange("p (r m) f -> p r m f", m=ghalf),
                            gout_fn(kk, 0)[:]
                            .rearrange("(r p) m f -> p r m f", p=128))
                        # readback#1 -> Zb in place (after the MM phase);
                        # sync ring, where waiting on AllGather#2 is benign
                        nc.sync.dma_start(
                            Zb[:].rearrange("p (r m) f -> p r m f", m=ghalf),
                            gout_fn(kk, 1)[:]
                            .rearrange("(r p) m f -> p r m f", p=128))

            # ================ conv1 =================
            with (
                tc.tile_pool(name="c1", bufs=1) as c1p,
                tc.tile_pool(name="w1s", bufs=2) as w1sp,
                tc.tile_pool(name="psz", bufs=2, space="PSUM") as pszp,
                tc.tile_pool(name="pst", bufs=2, space="PSUM") as pstp,
                tc.tile_pool(name="psg", bufs=2, space="PSUM") as psgp,
                tc.tile_pool(name="trsb", bufs=3) as trsbp,
            ):
                b1row = c1p.tile([1, B * G1], f16)
                nc.sync.dma_start(b1row[:], din['b1row'])
                Z1a = [c1p.tile([128, NBLK // 2, F1], f16, name=f"z1a{i}")
                       for i in range(2)]
                Z1b = c1p.tile([128, NBLK // 2, F1], f16)
                znl1 = [c1p.tile([128, MT, F1], f16, name=f"znl1_{i}")
                        for i in range(2)]
                nc.sync.dma_start(znl1[0][:], din['x_loc'])   # T_0 local
                acc1 = c1p.tile([128, MT, B * G1], f16)
                h1loc = c1p.tile([128, MT, B * G1], f16)
                nc.vector.memset(acc1[:], 0.0)
                g1i = [dramp.tile([128, MT // 2, F1], f16, name=f"g1i{h}")
                       for h in range(2)]
                g1o = {}

                def g1o_fn(kk, h):
                    if (kk, h) not in g1o:
                        g1o[(kk, h)] = dramp.tile(
                            [NC * 128, MT // 2, F1], f16,
                            addr_space="Shared", name=f"g1o_{kk}_{h}")
                    return g1o[(kk, h)]

                # stream the per-tap conv1 weights
                w1k = {}
                for kk in range(K):
                    t = w1sp.tile([8 * T, 8 * G1], f16, tag="w1k",
                                  name=f"w1k_{kk}")
                    nc.sync.dma_start(t[:], din['w1sb'][:, kk, :])
                    w1k[kk] = t

                # assemble the full T_0 = x state by gathering the local
                # shards (saves uploading a replicated full x per core)
                for h in range(2):
                    nc.sync.dma_start(
                        g1i[h][:], znl1[0][:, h * (MT // 2):(h + 1) * (MT // 2), :])
                    nc.gpsimd.collective_compute(
                        "AllGather", OP.bypass, replica_groups=RG,
                        ins=[g1i[h][:]], outs=[g1o_fn(-1, h)[:]])
                for h, dst in ((0, Z1a[0]), (1, Z1b)):
                    nc.sync.dma_start(
                        dst[:].rearrange("p (r m) f -> p r m f", m=MT // 2),
                        g1o_fn(-1, h)[:]
                        .rearrange("(r p) m f -> p r m f", p=128))

                with nc.named_scope("conv1"):
                    cheb_chunk(F1, Z1a, Z1b, znl1,
                               lambda kk: w1k[kk][:], 120,
                               b1row, 0, acc1, B * G1,
                               pszp, pstp, psgp, trsbp, g1i, g1o_fn, MT // 2)

                # h1 = relu(acc1 + b1) (bias already added), gather to DRAM
                nc.vector.tensor_scalar_max(h1loc[:], acc1[:], 0.0)
                if cfg.DEBUG:
                    nc.sync.dma_start(dbg_t1, znl1[0][:])   # T_24 local
                    nc.sync.dma_start(dbg_h1, h1loc[:])
                nc.sync.dma_start(gh1[:], h1loc[:])
                nc.gpsimd.collective_compute(
                    "AllGather", OP.bypass, replica_groups=RG,
                    ins=[gh1[:]], outs=[h1_dram[:]])

            # ================ conv2 (2 sequential chunks of 8 batches) =====
            p2 = tc.tile_pool(name="p2", bufs=1)
            p2p = p2.__enter__()
            acc2 = p2p.tile([128, MT, B * G2], f16)          # doubles as h2
            if cfg.PHASES < 2:
                zz = constp.tile([B, D], f32)
                nc.vector.memset(zz[:], 0.0)
                nc.sync.dma_start(dout, zz[:])
                p2.__exit__(None, None, None)
                return nc
            with (
                tc.tile_pool(name="c2", bufs=1) as c2p,
                tc.tile_pool(name="w2s", bufs=2) as w2sp,
                tc.tile_pool(name="psz2", bufs=2, space="PSUM") as psz2p,
                tc.tile_pool(name="pst2", bufs=2, space="PSUM") as pst2p,
                tc.tile_pool(name="psg2", bufs=2, space="PSUM") as psg2p,
                tc.tile_pool(name="trsb2", bufs=3) as trsb2p,
            ):
                b2row = c2p.tile([1, B * G2], f16)
                nc.sync.dma_start(b2row[:], din['b2row'])
                Z2a = [c2p.tile([128, NBLK // 2, F2], f16, name=f"z2a{i}")
                       for i in range(2)]
                Z2b = c2p.tile([128, NBLK // 2, F2], f16)
                znl2 = [c2p.tile([128, MT, F2], f16, name=f"znl2_{i}")
                        for i in range(2)]
                g2i = [dramp.tile([128, MT // 2, F2], f16, name=f"g2i{h}")
                       for h in range(2)]
                nc.vector.memset(acc2[:], 0.0)

                for ch in range(2):
                    fs = ch * F2
                    h1v = h1_dram[:].rearrange("(r p) m f -> r p m f", p=128)
                    for r in range(NC):
                        nc.sync.dma_start(
                            Z2a[0][:, r * (MT // 2):(r + 1) * (MT // 2), :],
                            h1v[r, :, 0:MT // 2, fs:fs + F2])
                        nc.sync.dma_start(
                            Z2b[:, r * (MT // 2):(r + 1) * (MT // 2), :],
                            h1v[r, :, MT // 2:MT, fs:fs + F2])
                    nc.sync.dma_start(znl2[0][:], gh1[:, :, fs:fs + F2])

                    # stream tap weights for this chunk
                    w2k = [None] * K
                    for kk in range(K):
                        w = w2sp.tile([4 * G1, 4 * G2], f16, tag="w2k",
                                      name=f"w2k_{ch}_{kk}")
                        nc.sync.dma_start(w[:], din['w2blk'][kk])
                        w2k[kk] = w

                    g2o = {}

                    def g2o_fn(kk, h, ch=ch, g2o=g2o):
                        if (kk, h) not in g2o:
                            g2o[(kk, h)] = dramp.tile(
                                [NC * 128, MT // 2, F2], f16,
                                addr_space="Shared",
                                name=f"g2o_{ch}_{kk}_{h}")
                        return g2o[(kk, h)]

                    with nc.named_scope(f"conv2_{ch}"):
                        cheb_chunk(F2, Z2a, Z2b, znl2,
                                   lambda kk: w2k[kk][:], 128,
                                   b2row, fs * 2, acc2, 2 * F2,
                                   psz2p, pst2p, psg2p, trsb2p,
                                   g2i, g2o_fn, MT // 2)

                # h2 = relu(acc2 + b2) in place (bias added in last tap)
                nc.vector.tensor_scalar_max(acc2[:], acc2[:], 0.0)
                if cfg.DEBUG:
                    nc.sync.dma_start(dbg_h2, acc2[:])

            # ================ fc1 (streamed weights) =================
            if cfg.PHASES < 3:
                zz = constp.tile([B, D], f32)
                nc.vector.memset(zz[:], 0.0)
                nc.sync.dma_start(dout, zz[:])
                p2.__exit__(None, None, None)
                return nc
            h2view = acc2[:].rearrange("p m (b g) -> p m b g", b=B)
            with (
                tc.tile_pool(name="fcw", bufs=2) as fcwp,
                tc.tile_pool(name="fcps", bufs=1, space="PSUM") as fcpsp,
                tc.tile_pool(name="fcsb", bufs=1) as fcsbp,
                tc.tile_pool(name="fcps2", bufs=2, space="PSUM") as fcps2p,
            ):
                JT = G2 * MT            # 512 j-tiles
                JBLK = 8
                psfc = fcpsp.tile([B, C], f32)
                fc1b_sb = fcsbp.tile([1, C], f16)
                nc.sync.dma_start(fc1b_sb[:], din['fc1b'])
                for jb in range(JT // JBLK):
                    wbuf8 = fcwp.tile([128, JBLK, C], i8, tag="wbuf8",
                                      bufs=4)
                    nc.sync.dma_start(wbuf8[:],
                                      din['wfc'][:, jb * JBLK:(jb + 1) * JBLK, :])
                    wbuf = fcwp.tile([128, JBLK, C], f16, tag="wbuf")
                    nc.vector.tensor_copy(wbuf[:], wbuf8[:])
                    for ji in range(JBLK):
                        jt = jb * JBLK + ji
                        g, mt = jt // MT, jt % MT
                        nc.tensor.matmul(psfc[:], h2view[:, mt, :, g],
                                         wbuf[:, ji, :],
                                         start=(jt == 0), stop=False)
                # bias via ones trick (last accumulation closes the group)
                nc.tensor.matmul(psfc[:], ones16[:1, :B], fc1b_sb[:1, :],
                                 start=False, stop=True)

                # transpose [B, C] -> [C/128 tiles of [128, B]]
                hsb = fcsbp.tile([B, C], f32)
                nc.vector.tensor_copy(hsb[:], psfc[:])
                hT = fcsbp.tile([128, C // 128, B], f32)
                for t4 in range(C // 128):
                    tp = fcps2p.tile([128, B], f32, tag="fct")
                    nc.tensor.transpose(tp[:], hsb[:, t4 * 128:(t4 + 1) * 128],
                                        identf32[:B, :B])
                    nc.vector.tensor_copy(hT[:, t4, :], tp[:])

                arin = dramp.tile([128, C // 128, B], f32)
                arout = dramp.tile([128, C // 128, B], f32,
                                   addr_space="Shared")
                nc.sync.dma_start(arin[:], hT[:])
                nc.gpsimd.collective_compute(
                    "AllReduce", OP.add, replica_groups=RG,
                    ins=[arin[:]], outs=[arout[:]])
                hTr = fcsbp.tile([128, C // 128, B], f32)
                nc.sync.dma_start(hTr[:], arout[:])

                # fc2: out[d, b] = fc2_w[d, :] @ h[:, b]
                fc2wt = fcsbp.tile([128, C // 128, D], f32)
                nc.sync.dma_start(fc2wt[:], din['fc2wt'])
                fc2b = fcsbp.tile([1, D], f32)
                nc.sync.dma_start(fc2b[:], din['fc2b'])
                onesf32 = fcsbp.tile([1, B], f32)
                nc.sync.dma_start(onesf32[:], din['onesf32'])
                ps2 = fcps2p.tile([D, B], f32, tag="ps2")
                for kt in range(C // 128):
                    nc.tensor.matmul(ps2[:], fc2wt[:, kt, :], hTr[:, kt, :],
                                     start=(kt == 0), stop=False)
                nc.tensor.matmul(ps2[:], fc2b[:1, :], onesf32[:1, :],
                                 start=False, stop=True)

                s2 = fcsbp.tile([D, B], f32)
                nc.vector.tensor_copy(s2[:], ps2[:])
                ps3 = fcps2p.tile([B, D], f32, tag="ps3")
                nc.tensor.transpose(ps3[:], s2[:], identf32[:D, :D])
                sm = fcsbp.tile([B, D], f32)
                nc.vector.tensor_copy(sm[:], ps3[:])

                # log_softmax over D (free axis)
                mx = fcsbp.tile([B, 1], f32)
                nc.vector.tensor_reduce(mx[:], sm[:], AX.X, OP.max)
                xm = fcsbp.tile([B, D], f32)
                nc.vector.tensor_single_scalar(xm[:], sm[:], mx[:], OP.subtract)
                ex = fcsbp.tile([B, D], f32)
                nc.scalar.activation(ex[:], xm[:], AT.Exp)
                sume = fcsbp.tile([B, 1], f32)
                nc.vector.tensor_reduce(sume[:], ex[:], AX.X, OP.add)
                lse = fcsbp.tile([B, 1], f32)
                nc.scalar.activation(lse[:], sume[:], AT.Ln)
                res = fcsbp.tile([B, D], f32)
                nc.vector.tensor_single_scalar(res[:], xm[:], lse[:],
                                               OP.subtract)
                nc.sync.dma_start(dout, res[:])
            p2.__exit__(None, None, None)

    return nc


def _run(cfg, inputs, trace=False):
    in_maps = _host_prep(cfg, **inputs)
    nc = _build(cfg)
    nc.compile()
    from concourse import bass_utils
    res = bass_utils.run_bass_kernel_spmd(
        nc, in_maps, core_ids=list(range(cfg.NCORES)), trace=trace)
    return np.asarray(res.results[0]['out'], np.float32).copy(), res


def kernel(**inputs):
    out, _ = _run(CFG(), inputs)
    return out
